# revision 5
# baseline (speedup 1.0000x reference)
"""Trainium2 Bass kernel for DCTTransform: 2D DCT -> 4 freq masks -> inverse DCT.

Strategy
--------
Data parallel over batch*channel (96 images of 512x512) across 8 cores
(12 images/core).  Per image, with D the orthonormal DCT-II matrix:

    Y   = D @ x @ D^T                  (2D DCT)
    out_i = D^T @ (Y * mask_i) @ D     (masked inverse)

Every matmul uses the *data* as the stationary (lhsT) operand and a
constant (D or D^T, fp16) as the streaming rhs.  Since the PE computes
lhsT.T @ rhs, each stage yields a transposed left-multiply, and chaining
four of them needs no explicit transposes:

    M1 = x.T @ D^T        -> (D @ x)^T          [s2, f1]
    M2 = M1.T @ D^T       -> Y                  [f1, f2]
    M3 = Ym.T @ D         -> (D^T @ Ym)^T       [f2, s1]
    M4 = M3.T @ D         -> out                [s1, s2]  (natural layout)

Structure exploited:
  * mask 3 (all ones): out == x exactly -> returned on host, no compute.
  * mask 0 (LH): support 128x128;  mask 1 (HL): 256x256 minus empty block;
    mask 2 (HH): anti-triangular -> ~40% of blocks zero.
  * Every partially-masked 128x128 block uses the same local anti-triangle.
fp16 keeps all operands (which are O(1)-scaled) at 1 cycle/row PE rate
with ~5e-4 relative error vs the fp32 reference.  I/O is fp16 on the
device side (x cast on host, outputs upcast on host) to halve DMA.
PSUM tiles span 2 banks so each PSUM->SBUF copy moves 2 matmul outputs;
output staging runs on the scalar engine, intermediate casts on vector.
"""

import sys

if "/opt/trn_rl_repo" not in sys.path:
    sys.path.insert(0, "/opt/trn_rl_repo")

import numpy as np

NCORES = 8
IMG = 512
P = 128
NT = IMG // P  # 4

# Masked 128x128 blocks (t=f1 tile, j=f2 tile) share one local anti-triangle
# mask; a block of mask with support S tiles is partial iff t + j == S - 1.
MASK_SPECS = (("lh", 1), ("hl", 2), ("hh", 4))
MASKED_BLOCKS = sorted(
    {(t, j) for _, S in MASK_SPECS for j in range(S) for t in range(S - j) if t + j == S - 1}
)


def build_program(nimg):
    import concourse.bacc as bacc
    import concourse.tile as tile
    import concourse.mybir as mybir

    f32, f16 = mybir.dt.float32, mybir.dt.float16

    nc = bacc.Bacc("TRN2", target_bir_lowering=False, debug=False, num_devices=NCORES)

    x_d = nc.dram_tensor("x", [nimg, IMG, IMG], f16, kind="ExternalInput")
    dm_d = nc.dram_tensor("dmat", [IMG, IMG], f16, kind="ExternalInput")
    dt_d = nc.dram_tensor("dmat_t", [IMG, IMG], f16, kind="ExternalInput")
    tri_d = nc.dram_tensor("tri", [P, P], f16, kind="ExternalInput")
    out_d = {
        nm: nc.dram_tensor(nm, [nimg, IMG, IMG], f16, kind="ExternalOutput")
        for nm, _ in MASK_SPECS
    }

    with tile.TileContext(nc) as tc:
        with (
            tc.tile_pool(name="const", bufs=1) as cpool,
            tc.tile_pool(name="io", bufs=3) as iopool,
            tc.tile_pool(name="work", bufs=2) as wpool,
            tc.tile_pool(name="blk", bufs=2) as bpool,
            tc.tile_pool(name="ps", bufs=4, space="PSUM") as pspool,
        ):
            cd = cpool.tile([P, NT, IMG], f16, tag="cd")  # D rows on partitions
            ct = cpool.tile([P, NT, IMG], f16, tag="ct")  # D^T rows on partitions
            tri = cpool.tile([P, P], f16, tag="tri")
            nc.sync.dma_start(cd[:], dm_d.rearrange("(t p) s -> p t s", p=P))
            nc.sync.dma_start(ct[:], dt_d.rearrange("(t p) s -> p t s", p=P))
            nc.sync.dma_start(tri[:], tri_d[:])

            for img in range(nimg):
                xb = iopool.tile([P, NT, IMG], f16, tag="xb")
                nc.sync.dma_start(xb[:], x_d[img].rearrange("(t p) s -> p t s", p=P))

                # M1 = x.T @ D^T; two 2-bank PSUM tiles, one cast each
                m1b = wpool.tile([P, NT, IMG], f16, tag="m1b")
                for h in range(2):
                    ps = pspool.tile([P, 2, IMG], f32, tag="ps")
                    for mr in range(2):
                        m = 2 * h + mr
                        for t in range(NT):
                            nc.tensor.matmul(
                                ps[:, mr, :], xb[:, t, P * m : P * (m + 1)], ct[:, t, :],
                                start=(t == 0), stop=(t == NT - 1),
                            )
                    nc.vector.tensor_copy(m1b[:, 2 * h : 2 * h + 2, :], ps[:])

                # M2 = M1.T @ D^T = Y
                y = wpool.tile([P, NT, IMG], f16, tag="y")
                for h in range(2):
                    ps = pspool.tile([P, 2, IMG], f32, tag="ps")
                    for mr in range(2):
                        m = 2 * h + mr
                        for t in range(NT):
                            nc.tensor.matmul(
                                ps[:, mr, :], m1b[:, t, P * m : P * (m + 1)], ct[:, t, :],
                                start=(t == 0), stop=(t == NT - 1),
                            )
                    nc.vector.tensor_copy(y[:, 2 * h : 2 * h + 2, :], ps[:])

                # Partial blocks: Y block (t,j) * anti-triangle
                tm = {}
                for (t, j) in MASKED_BLOCKS:
                    tmt = bpool.tile([P, P], f16, tag=f"tm{t}{j}")
                    nc.vector.tensor_mul(tmt[:], y[:, t, P * j : P * (j + 1)], tri[:])
                    tm[(t, j)] = tmt

                def blk(t, j, S):
                    if t + j == S - 1:
                        return tm[(t, j)][:]
                    return y[:, t, P * j : P * (j + 1)]

                for nm, S in MASK_SPECS:
                    # M3 = Ym.T @ D  -> V [f2, s1]
                    v = bpool.tile([P, S, IMG], f16, tag=f"v_{nm}")
                    nps = (S + 1) // 2  # psum tiles of up to 2 banks
                    for hp in range(nps):
                        js = [j for j in (2 * hp, 2 * hp + 1) if j < S]
                        ps = pspool.tile([P, len(js), IMG], f32, tag="ps")
                        for ji, j in enumerate(js):
                            ts = list(range(S - j))
                            for i, t in enumerate(ts):
                                nc.tensor.matmul(
                                    ps[:, ji, :], blk(t, j, S), cd[:, t, :],
                                    start=(i == 0), stop=(i == len(ts) - 1),
                                )
                        nc.vector.tensor_copy(v[:, 2 * hp : 2 * hp + len(js), :], ps[:])
                    # M4 = V.T @ D -> out [s1, s2]; stage on scalar engine
                    ot = iopool.tile([P, NT, IMG], f16, tag=f"ot_{nm}")
                    for h in range(2):
                        ps = pspool.tile([P, 2, IMG], f32, tag="ps")
                        for mr in range(2):
                            m = 2 * h + mr
                            for j in range(S):
                                nc.tensor.matmul(
                                    ps[:, mr, :], v[:, j, P * m : P * (m + 1)], cd[:, j, :],
                                    start=(j == 0), stop=(j == S - 1),
                                )
                        nc.scalar.copy(ot[:, 2 * h : 2 * h + 2, :], ps[:])
                    nc.sync.dma_start(
                        out_d[nm][img].rearrange("(t p) s -> p t s", p=P), ot[:]
                    )

    nc.compile()
    return nc


_prog_cache = {}

# Test-harness knobs (default off; the grading harness just calls kernel()).
TRACE = False
TRACE_KWARGS = {}
LAST_RESULTS = None


def _get_prog(nimg):
    if nimg not in _prog_cache:
        _prog_cache[nimg] = build_program(nimg)
    return _prog_cache[nimg]


def _dct_matrix_f16():
    k = np.arange(IMG, dtype=np.float64)[:, None]
    m = np.arange(IMG, dtype=np.float64)[None, :]
    D = np.cos(np.pi * (2.0 * m + 1.0) * k / (2.0 * IMG)) * np.sqrt(2.0 / IMG)
    D[0] *= 1.0 / np.sqrt(2.0)
    return D.astype(np.float16)


def kernel(x, masks):
    from concourse.bass_utils import run_bass_kernel_spmd

    x = np.ascontiguousarray(np.asarray(x), dtype=np.float32)
    masks = np.asarray(masks)
    B, C, H, W = x.shape
    n = B * C
    per = n // NCORES
    x16 = x.reshape(n, H, W).astype(np.float16)

    d16 = _dct_matrix_f16()
    dt16 = np.ascontiguousarray(d16.T)
    tri = np.ascontiguousarray(masks[0][:P, :P]).astype(np.float16)

    in_maps = [
        {
            "x": np.ascontiguousarray(x16[c * per : (c + 1) * per]),
            "dmat": d16,
            "dmat_t": dt16,
            "tri": tri,
        }
        for c in range(NCORES)
    ]

    nc = _get_prog(per)
    res = run_bass_kernel_spmd(
        nc, in_maps, list(range(NCORES)), trace=TRACE, **TRACE_KWARGS
    )
    global LAST_RESULTS
    LAST_RESULTS = res

    outs = {
        nm: np.concatenate([res.results[c][nm] for c in range(NCORES)], axis=0)
        .reshape(B, C, H, W)
        .astype(np.float32)
        for nm, _ in MASK_SPECS
    }
    LL = x.copy()
    return (LL, outs["lh"], outs["hl"], outs["hh"])
